# revision 6
# baseline (speedup 1.0000x reference)
"""DeepSeekMoE (H=1024, I=4096, E=8, top-2, T=16384) on 8 Trainium2 cores.

Strategy (expert parallelism, per the sharding hint):
  - Host computes router softmax/top-2 (tiny: T x E) with jax-on-CPU so the
    expert selection matches the reference bit-for-bit.
  - Core i holds routed expert i's weights and processes the tokens routed
    to expert i (gathered+padded on the host: the "all-to-all" is done
    host-side since full inputs arrive on the host).
  - Expert capacity is fixed at Q = T*TOP_K/E = 4096 tokens (capacity
    factor 1.0, the standard MoE design point). This makes every core's
    device work identical (4096 routed + 2048 shared columns = 12 full
    512-token tiles), removes all routing-imbalance padding waste, and
    makes the compiled program input-independent. The few tokens past an
    expert's capacity (~1.6% of pairs here) are computed exactly on the
    host in fp32 during the combine step, so the result is still exact.
  - The shared expert is data-parallel: core i also runs tokens
    [i*T/8, (i+1)*T/8) through the (replicated) shared expert.
  - Device computes MLPs in fp16 operands with fp32 PSUM accumulation in a
    transposed activation layout (hidden on partitions, tokens on the free
    dim), so no on-device transposes are needed anywhere. Outputs are
    written back in fp16 (the final combine accumulates in fp32 on host).
  - Host applies the top-2 routing weights and scatter-adds routed expert
    outputs back into token order (each token appears at most once per
    expert, so per-expert fancy-index += is collision-free).

The fp16 tensor engine is the bottleneck: 512 PE cycles per token column
(8 k-steps x 32 up-proj chunks + 32 x 8 down-proj) at 2.4 GHz. 6144
columns per core -> ~1.31 ms; the PE array runs at ~100% occupancy.
(fp8 DoubleRow was measured at 2x fp16 per unit K on this silicon, but
single-pass fp8 quantization noise is ~5% rel err vs the 2% budget, and
error-compensated 3-term fp8 costs 1.5x fp16 cycles - both lose.)
"""

import hashlib
import json
import os
import shutil

import numpy as np

H = 1024
I = 4096
E = 8
TOPK = 2
NCORES = 8
T = 16384
TS = T // NCORES  # shared-expert tokens per core
Q = T * TOPK // E  # routed-expert capacity per core (factor 1.0)
N = 512  # token tile (moving dim / one PSUM bank of fp32)

_NEFF_CACHE_DIR = os.path.join(
    os.path.expanduser("~"), ".cache", "bass_neff_cache"
)

_compiled = None  # finalized Bacc (fixed shapes; input-independent)
_cache_installed = False


def _install_neff_cache():
    """Cache walrus NEFF output by bir.json hash so repeated runs of the
    identical device program skip the multi-minute neuronxcc compile."""
    global _cache_installed
    if _cache_installed:
        return
    _cache_installed = True
    try:
        import concourse.bass_utils as bass_utils
        import concourse.bass2jax as bass2jax

        orig = bass_utils.compile_bir_kernel

        def canonical_key(bir_bytes):
            # The BIR embeds source paths/linenos (debug_table entries and
            # per-object ant_debug blobs). Strip those so the cache key only
            # reflects program semantics.
            try:
                m = json.loads(bir_bytes)
                m["debug_table"] = None
                stack = [m]
                while stack:
                    o = stack.pop()
                    if isinstance(o, dict):
                        o.pop("ant_debug", None)
                        stack.extend(o.values())
                    elif isinstance(o, list):
                        stack.extend(o)
                canon = json.dumps(m, sort_keys=True).encode()
            except Exception:
                canon = bir_bytes
            return hashlib.sha256(canon).hexdigest()

        def cached(bir_json, tmpdir, neff_name="file.neff"):
            if isinstance(bir_json, str):
                bir_bytes = bir_json.encode()
            else:
                bir_bytes = bir_json
            key = canonical_key(bir_bytes)
            cpath = os.path.join(_NEFF_CACHE_DIR, key + ".neff")
            dst = os.path.join(tmpdir, neff_name)
            if os.path.isfile(cpath):
                shutil.copyfile(cpath, dst)
                return dst
            out = orig(bir_json, tmpdir, neff_name)
            try:
                os.makedirs(_NEFF_CACHE_DIR, exist_ok=True)
                tmp = cpath + ".tmp%d" % os.getpid()
                shutil.copyfile(out, tmp)
                os.replace(tmp, cpath)
            except OSError:
                pass
            return out

        bass_utils.compile_bir_kernel = cached
        bass2jax.compile_bir_kernel = cached
    except Exception:
        pass


def _build():
    """Build the per-core SPMD device program (fixed shapes)."""
    import concourse.mybir as mybir
    import concourse.tile as tile
    from concourse import bacc

    f16 = mybir.dt.float16
    f32 = mybir.dt.float32
    silu = mybir.ActivationFunctionType.Silu

    nc = bacc.Bacc(None, target_bir_lowering=False)
    xs = nc.dram_tensor("xs", [H, TS], f16, kind="ExternalInput")
    xr = nc.dram_tensor("xr", [H, Q], f16, kind="ExternalInput")
    w1s = nc.dram_tensor("w1s", [H, I], f16, kind="ExternalInput")
    w2s = nc.dram_tensor("w2s", [I, H], f16, kind="ExternalInput")
    w1r = nc.dram_tensor("w1r", [H, I], f16, kind="ExternalInput")
    w2r = nc.dram_tensor("w2r", [I, H], f16, kind="ExternalInput")
    ys = nc.dram_tensor("ys", [H, TS], f16, kind="ExternalOutput")
    yr = nc.dram_tensor("yr", [H, Q], f16, kind="ExternalOutput")

    KT = H // 128  # 8 k-tiles over hidden
    IC = I // 128  # 32 i-chunks over intermediate
    HC = H // 128  # 8 output chunks over hidden

    with tile.TileContext(nc) as tc:
        with tc.tile_pool(name="wp", bufs=1) as wp, \
             tc.tile_pool(name="xp", bufs=2) as xp, \
             tc.tile_pool(name="hp", bufs=1) as hp, \
             tc.tile_pool(name="yp", bufs=3) as yp, \
             tc.tile_pool(name="pp", bufs=4, space="PSUM") as pp:

            def load_x(xT, t0, split=False):
                xt = xp.tile([128, KT, N], f16, tag="x")
                src = xT[:, t0:t0 + N].rearrange("(kt p) n -> p kt n", p=128)
                if split:
                    # per-k-tile DMAs so the first matmul only waits for
                    # the k=0 slice (128KB) instead of the full 1MB tile;
                    # k=0 itself is quartered across DMA queues (32KB each)
                    for q in range(4):
                        nc.sync.dma_start(out=xt[:, 0, q * 128:(q + 1) * 128],
                                          in_=src[:, 0, q * 128:(q + 1) * 128])
                    for k in range(1, KT):
                        nc.sync.dma_start(out=xt[:, k, :], in_=src[:, k, :])
                else:
                    nc.sync.dma_start(out=xt, in_=src)
                return xt

            def mlp(xT, w1, w2, yT, ntok, first=False):
                # first token tile load goes ahead of the weight streams
                xt0 = load_x(xT, 0, split=first)
                # weights striped into ~1MB DMAs: spreads across DMA queues
                # and lets the first matmuls start after ~1 stripe instead
                # of after the whole 8MB load; the leading stripes of the
                # first mlp are extra-fine so compute starts ASAP
                w1t = wp.tile([128, KT, I], f16, tag="w1")
                w1r_ap = w1.rearrange("(kt p) i -> p kt i", p=128)
                if first:
                    # the ic=0 weight block, one 32KB DMA per k-tile, so
                    # the first matmul group can start almost immediately
                    for k in range(KT):
                        nc.sync.dma_start(out=w1t[:, k, 0:128],
                                          in_=w1r_ap[:, k, 0:128])
                    bounds = [128, 256, 512] + list(range(1024, I + 1, 512))
                else:
                    bounds = list(range(0, I + 1, 512))
                for b0, b1 in zip(bounds, bounds[1:]):
                    sl = slice(b0, b1)
                    nc.sync.dma_start(out=w1t[:, :, sl], in_=w1r_ap[:, :, sl])
                w2t = wp.tile([128, IC, H], f16, tag="w2")
                w2r_ap = w2.rearrange("(it p) h -> p it h", p=128)
                for g in range(8):
                    sl = slice(g * (IC // 8), (g + 1) * (IC // 8))
                    nc.sync.dma_start(out=w2t[:, sl, :], in_=w2r_ap[:, sl, :])
                for t0 in range(0, ntok, N):
                    xt = xt0 if t0 == 0 else load_x(xT, t0)
                    ht = hp.tile([128, IC, N], f16, tag="h")
                    for ic in range(IC):
                        ps = pp.tile([128, N], f32, tag="hp")
                        for k in range(KT):
                            nc.tensor.matmul(
                                ps,
                                w1t[:, k, ic * 128:(ic + 1) * 128],
                                xt[:, k, :],
                                start=(k == 0),
                                stop=(k == KT - 1),
                            )
                        nc.scalar.activation(ht[:, ic, :], ps, silu)
                    for hc in range(HC):
                        yps = pp.tile([128, N], f32, tag="yp")
                        for ic in range(IC):
                            nc.tensor.matmul(
                                yps,
                                w2t[:, ic, hc * 128:(hc + 1) * 128],
                                ht[:, ic, :],
                                start=(ic == 0),
                                stop=(ic == IC - 1),
                            )
                        yt = yp.tile([128, N], f16, tag="y")
                        nc.vector.tensor_copy(yt, yps)
                        nc.sync.dma_start(
                            out=yT[hc * 128:(hc + 1) * 128, t0:t0 + N],
                            in_=yt,
                        )

            mlp(xs, w1s, w2s, ys, TS, first=True)
            mlp(xr, w1r, w2r, yr, Q)

    nc.finalize()
    return nc


def _get_nc():
    global _compiled
    if _compiled is None:
        _compiled = _build()
    return _compiled


# test-harness knobs (ignored in normal use)
TRACE = False
LAST_RESULT = None


def _silu(v):
    return v / (1.0 + np.exp(-v))


def kernel(hidden_states, w1_shared, w2_shared, w1_routed, w2_routed,
           w_router):
    import jax
    from concourse.bass_utils import run_bass_kernel_spmd

    _install_neff_cache()

    hidden_states = np.asarray(hidden_states, dtype=np.float32)
    w_router = np.asarray(w_router, dtype=np.float32)
    flat = np.ascontiguousarray(hidden_states.reshape(-1, H))

    # --- routing on host, bit-identical to the reference (jax on CPU) ---
    cpu = jax.devices("cpu")[0]
    with jax.default_device(cpu):
        jflat = jax.device_put(flat, cpu)
        jrouter = jax.device_put(w_router, cpu)
        logits = jflat @ jrouter
        rw = jax.nn.softmax(logits, axis=-1)
        topw, topi = jax.lax.top_k(rw, TOPK)
        topw = topw / jax.numpy.sum(topw, axis=-1, keepdims=True)
    topw = np.asarray(topw)  # [T, K] f32
    topi = np.asarray(topi)  # [T, K] int32

    pairs_e = topi.ravel()  # expert of each (token, k) slot
    order = np.argsort(pairs_e, kind="stable")
    counts = np.bincount(pairs_e, minlength=E)
    starts = np.zeros(E + 1, np.int64)
    np.cumsum(counts, out=starts[1:])
    tok_by_e = [order[starts[e]:starts[e + 1]] // TOPK for e in range(E)]
    w_by_e = [topw.ravel()[order[starts[e]:starts[e + 1]]] for e in range(E)]

    # --- build per-core inputs (fp16, transposed activations) ---
    flatT16 = np.ascontiguousarray(flat.T.astype(np.float16))  # [H, T]
    w1s16 = np.asarray(w1_shared, dtype=np.float16)
    w2s16 = np.asarray(w2_shared, dtype=np.float16)
    w1r16 = np.asarray(w1_routed, dtype=np.float16)
    w2r16 = np.asarray(w2_routed, dtype=np.float16)

    in_maps = []
    for i in range(NCORES):
        nd = min(int(counts[i]), Q)  # device-resident tokens for expert i
        xr_i = np.zeros((H, Q), np.float16)
        xr_i[:, :nd] = flatT16[:, tok_by_e[i][:nd]]
        in_maps.append({
            "xs": np.ascontiguousarray(flatT16[:, i * TS:(i + 1) * TS]),
            "xr": xr_i,
            "w1s": w1s16,
            "w2s": w2s16,
            "w1r": w1r16[i],
            "w2r": w2r16[i],
        })

    nc = _get_nc()
    try:
        res = run_bass_kernel_spmd(nc, in_maps, list(range(NCORES)),
                                   trace=TRACE)
    except Exception:
        # transient NRT/device hiccups have been observed to clear on retry
        res = run_bass_kernel_spmd(nc, in_maps, list(range(NCORES)),
                                   trace=TRACE)
    global LAST_RESULT
    LAST_RESULT = res

    # --- combine on host (fp32 accumulation) ---
    total = np.empty((T, H), np.float32)
    for i in range(NCORES):
        total[i * TS:(i + 1) * TS] = res.results[i]["ys"].T
    routed = np.zeros((T, H), np.float32)
    w1r32 = np.asarray(w1_routed, dtype=np.float32)
    w2r32 = np.asarray(w2_routed, dtype=np.float32)
    for e in range(E):
        ne = int(counts[e])
        nd = min(ne, Q)
        if nd:
            ye = res.results[e]["yr"][:, :nd].T.astype(np.float32)
            routed[tok_by_e[e][:nd]] += w_by_e[e][:nd, None] * ye
        if ne > Q:
            # capacity overflow: exact host fp32 compute for the few
            # tokens past this expert's device capacity
            toks = tok_by_e[e][Q:]
            yo = _silu(flat[toks] @ w1r32[e]) @ w2r32[e]
            routed[toks] += w_by_e[e][Q:, None] * yo
    total += routed
    return total.reshape(hidden_states.shape)


# revision 7
# speedup vs baseline: 1.0039x; 1.0039x over previous
"""DeepSeekMoE (H=1024, I=4096, E=8, top-2, T=16384) on 8 Trainium2 cores.

Strategy (expert parallelism, per the sharding hint):
  - Host computes router softmax/top-2 (tiny: T x E) with jax-on-CPU so the
    expert selection matches the reference bit-for-bit.
  - Core i holds routed expert i's weights and processes the tokens routed
    to expert i (gathered+padded on the host: the "all-to-all" is done
    host-side since full inputs arrive on the host).
  - Expert capacity is fixed at Q = T*TOP_K/E = 4096 tokens (capacity
    factor 1.0, the standard MoE design point). This makes every core's
    device work identical (4096 routed + 2048 shared columns = 12 full
    512-token tiles), removes all routing-imbalance padding waste, and
    makes the compiled program input-independent. The few tokens past an
    expert's capacity (~1.6% of pairs here) are computed exactly on the
    host in fp32 during the combine step, so the result is still exact.
  - The shared expert is data-parallel: core i also runs tokens
    [i*T/8, (i+1)*T/8) through the (replicated) shared expert.
  - Device computes MLPs in fp16 operands with fp32 PSUM accumulation in a
    transposed activation layout (hidden on partitions, tokens on the free
    dim), so no on-device transposes are needed anywhere. Outputs are
    written back in fp16 (the final combine accumulates in fp32 on host).
  - Host applies the top-2 routing weights and scatter-adds routed expert
    outputs back into token order (each token appears at most once per
    expert, so per-expert fancy-index += is collision-free).

The fp16 tensor engine is the bottleneck: 512 PE cycles per token column
(8 k-steps x 32 up-proj chunks + 32 x 8 down-proj) at 2.4 GHz. 6144
columns per core -> ~1.31 ms; the PE array runs at ~100% occupancy.
(fp8 DoubleRow was measured at 2x fp16 per unit K on this silicon, but
single-pass fp8 quantization noise is ~5% rel err vs the 2% budget, and
error-compensated 3-term fp8 costs 1.5x fp16 cycles - both lose.)
"""

import hashlib
import json
import os
import shutil

import numpy as np

H = 1024
I = 4096
E = 8
TOPK = 2
NCORES = 8
T = 16384
TS = T // NCORES  # shared-expert tokens per core
Q = T * TOPK // E  # routed-expert capacity per core (factor 1.0)
N = 512  # token tile (moving dim / one PSUM bank of fp32)

_NEFF_CACHE_DIR = os.path.join(
    os.path.expanduser("~"), ".cache", "bass_neff_cache"
)

_compiled = None  # finalized Bacc (fixed shapes; input-independent)
_cache_installed = False


def _install_neff_cache():
    """Cache walrus NEFF output by bir.json hash so repeated runs of the
    identical device program skip the multi-minute neuronxcc compile."""
    global _cache_installed
    if _cache_installed:
        return
    _cache_installed = True
    try:
        import concourse.bass_utils as bass_utils
        import concourse.bass2jax as bass2jax

        orig = bass_utils.compile_bir_kernel

        def canonical_key(bir_bytes):
            # The BIR embeds source paths/linenos (debug_table entries and
            # per-object ant_debug blobs). Strip those so the cache key only
            # reflects program semantics.
            try:
                m = json.loads(bir_bytes)
                m["debug_table"] = None
                stack = [m]
                while stack:
                    o = stack.pop()
                    if isinstance(o, dict):
                        o.pop("ant_debug", None)
                        stack.extend(o.values())
                    elif isinstance(o, list):
                        stack.extend(o)
                canon = json.dumps(m, sort_keys=True).encode()
            except Exception:
                canon = bir_bytes
            return hashlib.sha256(canon).hexdigest()

        def cached(bir_json, tmpdir, neff_name="file.neff"):
            if isinstance(bir_json, str):
                bir_bytes = bir_json.encode()
            else:
                bir_bytes = bir_json
            key = canonical_key(bir_bytes)
            cpath = os.path.join(_NEFF_CACHE_DIR, key + ".neff")
            dst = os.path.join(tmpdir, neff_name)
            if os.path.isfile(cpath):
                shutil.copyfile(cpath, dst)
                return dst
            out = orig(bir_json, tmpdir, neff_name)
            try:
                os.makedirs(_NEFF_CACHE_DIR, exist_ok=True)
                tmp = cpath + ".tmp%d" % os.getpid()
                shutil.copyfile(out, tmp)
                os.replace(tmp, cpath)
            except OSError:
                pass
            return out

        bass_utils.compile_bir_kernel = cached
        bass2jax.compile_bir_kernel = cached
    except Exception:
        pass


def _build():
    """Build the per-core SPMD device program (fixed shapes)."""
    import concourse.mybir as mybir
    import concourse.tile as tile
    from concourse import bacc

    f16 = mybir.dt.float16
    f32 = mybir.dt.float32
    silu = mybir.ActivationFunctionType.Silu

    nc = bacc.Bacc(None, target_bir_lowering=False)
    xs = nc.dram_tensor("xs", [H, TS], f16, kind="ExternalInput")
    xr = nc.dram_tensor("xr", [H, Q], f16, kind="ExternalInput")
    w1s = nc.dram_tensor("w1s", [H, I], f16, kind="ExternalInput")
    w2s = nc.dram_tensor("w2s", [I, H], f16, kind="ExternalInput")
    w1r = nc.dram_tensor("w1r", [H, I], f16, kind="ExternalInput")
    w2r = nc.dram_tensor("w2r", [I, H], f16, kind="ExternalInput")
    ys = nc.dram_tensor("ys", [H, TS], f16, kind="ExternalOutput")
    yr = nc.dram_tensor("yr", [H, Q], f16, kind="ExternalOutput")

    KT = H // 128  # 8 k-tiles over hidden
    IC = I // 128  # 32 i-chunks over intermediate
    HC = H // 128  # 8 output chunks over hidden

    with tile.TileContext(nc) as tc:
        with tc.tile_pool(name="wp", bufs=1) as wp, \
             tc.tile_pool(name="xp", bufs=2) as xp, \
             tc.tile_pool(name="hp", bufs=1) as hp, \
             tc.tile_pool(name="yp", bufs=3) as yp, \
             tc.tile_pool(name="pp", bufs=2, space="PSUM") as pp:

            def load_x(xT, t0):
                xt = xp.tile([128, KT, N], f16, tag="x")
                nc.sync.dma_start(
                    out=xt,
                    in_=xT[:, t0:t0 + N].rearrange("(kt p) n -> p kt n", p=128),
                )
                return xt

            def mlp(xT, w1, w2, yT, ntok):
                # first token tile load goes ahead of the weight streams
                xt0 = load_x(xT, 0)
                # weights striped into 1MB DMAs: spreads across DMA queues
                # and lets the first matmuls start after ~1 stripe instead
                # of after the whole 8MB load
                w1t = wp.tile([128, KT, I], f16, tag="w1")
                w1r_ap = w1.rearrange("(kt p) i -> p kt i", p=128)
                for g in range(8):
                    sl = slice(g * (I // 8), (g + 1) * (I // 8))
                    nc.sync.dma_start(out=w1t[:, :, sl], in_=w1r_ap[:, :, sl])
                w2t = wp.tile([128, IC, H], f16, tag="w2")
                w2r_ap = w2.rearrange("(it p) h -> p it h", p=128)
                for g in range(8):
                    sl = slice(g * (IC // 8), (g + 1) * (IC // 8))
                    nc.sync.dma_start(out=w2t[:, sl, :], in_=w2r_ap[:, sl, :])
                for t0 in range(0, ntok, N):
                    xt = xt0 if t0 == 0 else load_x(xT, t0)
                    ht = hp.tile([128, IC, N], f16, tag="h")
                    for ic in range(IC):
                        ps = pp.tile([128, N], f32, tag="hp")
                        for k in range(KT):
                            nc.tensor.matmul(
                                ps,
                                w1t[:, k, ic * 128:(ic + 1) * 128],
                                xt[:, k, :],
                                start=(k == 0),
                                stop=(k == KT - 1),
                            )
                        nc.scalar.activation(ht[:, ic, :], ps, silu)
                    for hc in range(HC):
                        yps = pp.tile([128, N], f32, tag="yp")
                        for ic in range(IC):
                            nc.tensor.matmul(
                                yps,
                                w2t[:, ic, hc * 128:(hc + 1) * 128],
                                ht[:, ic, :],
                                start=(ic == 0),
                                stop=(ic == IC - 1),
                            )
                        yt = yp.tile([128, N], f16, tag="y")
                        nc.vector.tensor_copy(yt, yps)
                        nc.sync.dma_start(
                            out=yT[hc * 128:(hc + 1) * 128, t0:t0 + N],
                            in_=yt,
                        )

            mlp(xs, w1s, w2s, ys, TS)
            mlp(xr, w1r, w2r, yr, Q)

    nc.finalize()
    return nc


def _get_nc():
    global _compiled
    if _compiled is None:
        _compiled = _build()
    return _compiled


# test-harness knobs (ignored in normal use)
TRACE = False
LAST_RESULT = None


def _silu(v):
    return v / (1.0 + np.exp(-v))


def kernel(hidden_states, w1_shared, w2_shared, w1_routed, w2_routed,
           w_router):
    import jax
    from concourse.bass_utils import run_bass_kernel_spmd

    _install_neff_cache()

    hidden_states = np.asarray(hidden_states, dtype=np.float32)
    w_router = np.asarray(w_router, dtype=np.float32)
    flat = np.ascontiguousarray(hidden_states.reshape(-1, H))

    # --- routing on host, bit-identical to the reference (jax on CPU) ---
    cpu = jax.devices("cpu")[0]
    with jax.default_device(cpu):
        jflat = jax.device_put(flat, cpu)
        jrouter = jax.device_put(w_router, cpu)
        logits = jflat @ jrouter
        rw = jax.nn.softmax(logits, axis=-1)
        topw, topi = jax.lax.top_k(rw, TOPK)
        topw = topw / jax.numpy.sum(topw, axis=-1, keepdims=True)
    topw = np.asarray(topw)  # [T, K] f32
    topi = np.asarray(topi)  # [T, K] int32

    pairs_e = topi.ravel()  # expert of each (token, k) slot
    order = np.argsort(pairs_e, kind="stable")
    counts = np.bincount(pairs_e, minlength=E)
    starts = np.zeros(E + 1, np.int64)
    np.cumsum(counts, out=starts[1:])
    tok_by_e = [order[starts[e]:starts[e + 1]] // TOPK for e in range(E)]
    w_by_e = [topw.ravel()[order[starts[e]:starts[e + 1]]] for e in range(E)]

    # --- build per-core inputs (fp16, transposed activations) ---
    flatT16 = np.ascontiguousarray(flat.T.astype(np.float16))  # [H, T]
    w1s16 = np.asarray(w1_shared, dtype=np.float16)
    w2s16 = np.asarray(w2_shared, dtype=np.float16)
    w1r16 = np.asarray(w1_routed, dtype=np.float16)
    w2r16 = np.asarray(w2_routed, dtype=np.float16)

    in_maps = []
    for i in range(NCORES):
        nd = min(int(counts[i]), Q)  # device-resident tokens for expert i
        xr_i = np.zeros((H, Q), np.float16)
        xr_i[:, :nd] = flatT16[:, tok_by_e[i][:nd]]
        in_maps.append({
            "xs": np.ascontiguousarray(flatT16[:, i * TS:(i + 1) * TS]),
            "xr": xr_i,
            "w1s": w1s16,
            "w2s": w2s16,
            "w1r": w1r16[i],
            "w2r": w2r16[i],
        })

    nc = _get_nc()
    try:
        res = run_bass_kernel_spmd(nc, in_maps, list(range(NCORES)),
                                   trace=TRACE)
    except Exception:
        # transient NRT/device hiccups have been observed to clear on retry
        res = run_bass_kernel_spmd(nc, in_maps, list(range(NCORES)),
                                   trace=TRACE)
    global LAST_RESULT
    LAST_RESULT = res

    # --- combine on host (fp32 accumulation) ---
    total = np.empty((T, H), np.float32)
    for i in range(NCORES):
        total[i * TS:(i + 1) * TS] = res.results[i]["ys"].T
    routed = np.zeros((T, H), np.float32)
    w1r32 = np.asarray(w1_routed, dtype=np.float32)
    w2r32 = np.asarray(w2_routed, dtype=np.float32)
    for e in range(E):
        ne = int(counts[e])
        nd = min(ne, Q)
        if nd:
            ye = res.results[e]["yr"][:, :nd].T.astype(np.float32)
            routed[tok_by_e[e][:nd]] += w_by_e[e][:nd, None] * ye
        if ne > Q:
            # capacity overflow: exact host fp32 compute for the few
            # tokens past this expert's device capacity
            toks = tok_by_e[e][Q:]
            yo = _silu(flat[toks] @ w1r32[e]) @ w2r32[e]
            routed[toks] += w_by_e[e][Q:, None] * yo
    total += routed
    return total.reshape(hidden_states.shape)
